# revision 11
# baseline (speedup 1.0000x reference)
"""Trainium2 Bass kernel for nn_AttentionBlock (GroupNorm + attention block),
data-parallel over batch across 8 NeuronCores.

Reference computation (per batch element b, C=512, N=H*W=1024, heads=8, hd=64):
  xn   = GroupNorm32(x) * gamma + beta
  qkv  = w_qkv @ xn + b_qkv        (1x1 conv == matmul over channels)
  attn = softmax(q^T k / sqrt(hd)) ; ha = attn @ v ; out = x + w_proj @ ha + b_proj

Sharding: batch B=8 -> one batch element per core. No collectives.

Per-core dataflow (matmuls in bf16 with f32 PSUM accumulation; weights are
passed as f32 and converted to bf16 on chip):
  - GroupNorm stats in f32: per-channel sum / sum-of-squares on DVE, group
    reduction + per-channel re-broadcast via tiny TensorE matmuls,
    rsqrt(var+eps) computed as exp(-0.5*ln(var+eps)) to stay in a single
    ScalarE table set (with the softmax Exp).
  - q,k produced in [channel, n] layout with host-pretransposed w_qkT.
  - v produced directly TRANSPOSED, v_T[n, c_v] = xn^T @ w_v^T (host
    pretransposed rhs), so attention needs no on-chip transposes. The v bias
    is folded in via a ones-row matmul accumulation. v_T is stored with
    head stride 65: 64 v columns + a ones column per head.
  - Scores computed transposed: S_T[m, n] = k_h^T q_h (K = hd = 64).
    Softmax without max subtraction (|scores*0.125| <~ 8, exp is safe in f32):
    P_T = exp(S_T * 0.125) on ScalarE directly out of PSUM (doubles as the
    PSUM eviction), written as bf16.
  - AV: ha_u[65, n] = [v_T | ones]^T @ P_T accumulated over the 8 m-chunks;
    row 64 is the softmax denominator Z. Normalization by 1/Z is applied at
    eviction: reciprocal on DVE, broadcast over partitions via a K=1 ones
    matmul on TensorE, multiply on DVE (writing bf16 for the proj matmul).
  - proj: out = (psum + b_proj) + x fused in one DVE scalar_tensor_tensor.
"""

import os

import numpy as np

import concourse.bass as bass
import concourse.bacc as bacc
import concourse.mybir as mybir
import concourse.tile as tile
from concourse.bass_utils import run_bass_kernel_spmd

F32 = mybir.dt.float32
BF16 = mybir.dt.bfloat16
AF = mybir.ActivationFunctionType
ALU = mybir.AluOpType

B = 8
C = 512
N = 1024          # H*W = 32*32
H = 8             # num heads
HD = 64           # head dim
G = 32            # groups
GS = C // G       # channels per group = 16
CCH = 4           # channel chunks of 128
NT = 2            # n tiles of 512
MT = 8            # m tiles of 128
EPS = 1e-5
P = 128
NCORES = 8

_CACHE = {}


def build_nc():
    nc = bacc.Bacc(
        "TRN2", target_bir_lowering=False, debug=False, num_devices=NCORES
    )

    # All parameters are 2-D float32, pre-arranged on the host so every DMA
    # below is a plain contiguous copy.
    x_d = nc.declare_dram_parameter("x", [C, N], F32, isOutput=False)
    wqk_d = nc.declare_dram_parameter("w_qkT", [C, 2 * C], F32, isOutput=False)
    bqk_d = nc.declare_dram_parameter("b_qk", [P, 8], F32, isOutput=False)
    wv_d = nc.declare_dram_parameter("w_vT", [C + 1, C], F32, isOutput=False)
    wp_d = nc.declare_dram_parameter("w_pT", [C, C], F32, isOutput=False)
    bp_d = nc.declare_dram_parameter("b_p", [P, CCH], F32, isOutput=False)
    gm_d = nc.declare_dram_parameter("gamma", [P, CCH], F32, isOutput=False)
    bt_d = nc.declare_dram_parameter("beta", [P, CCH], F32, isOutput=False)
    gsel_d = nc.declare_dram_parameter("gsel", [P, 8], F32, isOutput=False)
    gselT_d = nc.declare_dram_parameter("gselT", [8, P], F32, isOutput=False)
    out_d = nc.declare_dram_parameter("out", [C, N], F32, isOutput=True)

    with tile.TileContext(nc) as tc:
        with (
            tc.tile_pool(name="singles", bufs=1) as singles,
            tc.tile_pool(name="wstage", bufs=2) as wstage,
            tc.tile_pool(name="ps", bufs=3, space="PSUM") as ps_pool,
            tc.tile_pool(name="ps_av", bufs=1, space="PSUM") as ps_av_pool,
        ):
            # ---------------- static tiles ----------------
            x_sb = singles.tile([P, CCH, N], F32)
            wqk_sb = singles.tile([P, CCH, 2 * C], BF16)
            bqk_sb = singles.tile([P, 8], F32)
            wv_sb = singles.tile([P, CCH, C], BF16)
            wvb_sb = singles.tile([1, C], BF16)
            wp_sb = singles.tile([P, CCH, C], BF16)
            bp_sb = singles.tile([P, CCH], F32)
            gm_sb = singles.tile([P, CCH], F32)
            bt_sb = singles.tile([P, CCH], F32)
            gsel_sb = singles.tile([P, 8], BF16)
            gselT_sb = singles.tile([8, P], BF16)
            gsel_st = singles.tile([P, 8], F32)
            gselT_st = singles.tile([8, P], F32)
            s12_bf = singles.tile([P, 8], BF16)
            mu_rs_bf = singles.tile([8, 8], BF16)
            ones_row = singles.tile([1, P], BF16)
            ones64 = singles.tile([1, HD], BF16)

            xn_sb = singles.tile([P, CCH, N], BF16)
            qk_sb = singles.tile([P, 8, N], BF16)       # ot 0-3: q, 4-7: k
            vT_sb = singles.tile([P, MT, H * (HD + 1)], BF16)
            pT_a = singles.tile([P, MT, N], BF16)
            pT_b = singles.tile([P, MT, N], BF16)
            ha_sb = singles.tile([P, CCH, N], BF16)
            out_sb = singles.tile([P, CCH, N], F32)

            s12_sb = singles.tile([P, 8], F32)          # cols 0-3 sum, 4-7 sumsq
            sq_scr = singles.tile([P, N], F32)
            mu_rs = singles.tile([8, 8], F32)           # cols 0-3 mu, 4-7 rs
            ex2_sb = singles.tile([8, CCH], F32)
            tmp8 = singles.tile([8, CCH], F32)
            var_sb = singles.tile([8, CCH], F32)
            lnv_sb = singles.tile([8, CCH], F32)
            s0_sb = singles.tile([P, CCH], F32)
            sbias_sb = singles.tile([P, CCH], F32)
            tmp128 = singles.tile([P, CCH], F32)
            eps_sb = singles.tile([8, 1], F32)
            zinv_t = [
                singles.tile([1, N], BF16, name=f"zinv{i}") for i in range(2)
            ]
            zln = singles.tile([1, N], F32)
            zeps_sb = singles.tile([1, 1], F32)
            zb_t = [singles.tile([HD, N], F32, name=f"zb{i}") for i in range(2)]

            # selector matrices DMA'd from host, converted to bf16 on chip
            nc.sync.dma_start(gsel_st[:], gsel_d.ap())
            nc.sync.dma_start(gselT_st[:], gselT_d.ap())
            nc.vector.tensor_copy(gsel_sb[:], gsel_st[:])
            nc.vector.tensor_copy(gselT_sb[:], gselT_st[:])
            nc.vector.memset(ones_row[:], 1.0)
            nc.vector.memset(ones64[:], 1.0)
            nc.vector.memset(eps_sb[:], EPS)
            nc.vector.memset(zeps_sb[:], 0.0)

            # ---------------- input DMAs + weight bf16 conversion ----------
            x_v = x_d.ap().rearrange("(cc p) n -> p cc n", p=P)
            for cc in range(CCH):
                nc.sync.dma_start(x_sb[:, cc, :], x_v[:, cc, :])

            wqk_st = wstage.tile([P, CCH, 2 * C], F32, tag="wst")
            nc.sync.dma_start(
                wqk_st[:], wqk_d.ap().rearrange("(cc p) o -> p cc o", p=P)
            )
            nc.vector.tensor_copy(wqk_sb[:], wqk_st[:])

            wv_st = wstage.tile([P, CCH, C], F32, tag="wst")
            nc.sync.dma_start(
                wv_st[:], wv_d.ap()[0:C, :].rearrange("(cc p) v -> p cc v", p=P)
            )
            nc.vector.tensor_copy(wv_sb[:], wv_st[:])
            wvb_st = wstage.tile([1, C], F32, tag="wvbst")
            nc.sync.dma_start(wvb_st[:], wv_d.ap()[C : C + 1, :])
            nc.vector.tensor_copy(wvb_sb[:], wvb_st[:])

            wp_st = wstage.tile([P, CCH, C], F32, tag="wst")
            nc.sync.dma_start(
                wp_st[:], wp_d.ap().rearrange("(cc p) o -> p cc o", p=P)
            )
            nc.vector.tensor_copy(wp_sb[:], wp_st[:])

            nc.sync.dma_start(bqk_sb[:], bqk_d.ap())
            nc.sync.dma_start(bp_sb[:], bp_d.ap())
            nc.sync.dma_start(gm_sb[:], gm_d.ap())
            nc.sync.dma_start(bt_sb[:], bt_d.ap())

            # ---------------- GroupNorm stats ----------------
            for cc in range(CCH):
                nc.vector.reduce_sum(
                    s12_sb[:, cc : cc + 1], x_sb[:, cc, :], axis=mybir.AxisListType.X
                )
                nc.vector.tensor_mul(sq_scr[:], x_sb[:, cc, :], x_sb[:, cc, :])
                nc.vector.reduce_sum(
                    s12_sb[:, 4 + cc : 5 + cc], sq_scr[:], axis=mybir.AxisListType.X
                )
            # group reduce: [8 groups-in-chunk, 8 (s1 x cc, s2 x cc)]
            nc.vector.tensor_copy(s12_bf[:], s12_sb[:])
            ps_st = ps_pool.tile([P, N], F32, tag="ps")
            nc.tensor.matmul(
                ps_st[0:8, 0:8], gsel_sb[:], s12_bf[:], start=True, stop=True
            )
            inv_cnt = 1.0 / (GS * N)
            nc.vector.tensor_scalar_mul(mu_rs[:, 0:4], ps_st[0:8, 0:4], inv_cnt)
            nc.vector.tensor_scalar_mul(ex2_sb[:], ps_st[0:8, 4:8], inv_cnt)
            nc.vector.tensor_mul(tmp8[:], mu_rs[:, 0:4], mu_rs[:, 0:4])
            nc.vector.tensor_sub(var_sb[:], ex2_sb[:], tmp8[:])
            # rsqrt(var+eps) = exp(-0.5 * ln(var+eps)); keeps one ACT table set
            nc.scalar.activation(lnv_sb[:], var_sb[:], AF.Ln, bias=eps_sb[:])
            nc.scalar.activation(mu_rs[:, 4:8], lnv_sb[:], AF.Exp, scale=-0.5)
            # broadcast group stats back to channels
            nc.vector.tensor_copy(mu_rs_bf[:], mu_rs[:])
            ps_bc = ps_pool.tile([P, N], F32, tag="ps")
            nc.tensor.matmul(
                ps_bc[0:P, 0:8], gselT_sb[:], mu_rs_bf[:], start=True, stop=True
            )
            nc.vector.tensor_mul(s0_sb[:], ps_bc[0:P, 4:8], gm_sb[:])
            nc.vector.tensor_mul(tmp128[:], ps_bc[0:P, 0:4], s0_sb[:])
            nc.vector.tensor_sub(sbias_sb[:], bt_sb[:], tmp128[:])
            # xn = x * s0 + sbias  (bf16)
            for cc in range(CCH):
                nc.vector.tensor_scalar(
                    out=xn_sb[:, cc, :],
                    in0=x_sb[:, cc, :],
                    scalar1=s0_sb[:, cc : cc + 1],
                    scalar2=sbias_sb[:, cc : cc + 1],
                    op0=ALU.mult,
                    op1=ALU.add,
                )

            # ---------------- q, k ----------------
            for ot in range(8):
                ps_qk = ps_pool.tile([P, N], F32, tag="ps")
                for nt in range(NT):
                    for cc in range(CCH):
                        nc.tensor.matmul(
                            ps_qk[:, nt * 512 : (nt + 1) * 512],
                            wqk_sb[:, cc, ot * P : (ot + 1) * P],
                            xn_sb[:, cc, nt * 512 : (nt + 1) * 512],
                            start=(cc == 0),
                            stop=(cc == CCH - 1),
                        )
                nc.vector.tensor_scalar_add(
                    qk_sb[:, ot, :], ps_qk[:], bqk_sb[:, ot : ot + 1]
                )

            # ---------------- v_T (+ bias via ones row) ----------------
            for mt in range(MT):
                ps_v = ps_pool.tile([P, N], F32, tag="ps")
                for cc in range(CCH):
                    nc.tensor.matmul(
                        ps_v[:, 0:C],
                        xn_sb[:, cc, mt * P : (mt + 1) * P],
                        wv_sb[:, cc, :],
                        start=(cc == 0),
                        stop=False,
                    )
                nc.tensor.matmul(
                    ps_v[:, 0:C], ones_row[:], wvb_sb[:], start=False, stop=True
                )
                nc.vector.tensor_copy(
                    vT_sb[:, mt, :]
                    .rearrange("p (h d) -> p h d", h=H)[:, :, 0:HD],
                    ps_v[:, 0:C].rearrange("p (h d) -> p h d", h=H),
                )
            nc.vector.memset(
                vT_sb[:].rearrange("p mt (h d) -> p mt h d", h=H)[:, :, :, HD : HD + 1],
                1.0,
            )

            # ---------------- attention per head ----------------
            for h in range(H):
                pT = pT_a if h % 2 == 0 else pT_b
                po = (h % 2) * HD
                ot = h // 2
                zinv = zinv_t[h % 2]
                zb = zb_t[h % 2]
                for mt in range(MT):
                    ps_s = ps_pool.tile([P, N], F32, tag="ps")
                    for nt in range(NT):
                        nc.tensor.matmul(
                            ps_s[:, nt * 512 : (nt + 1) * 512],
                            qk_sb[po : po + HD, 4 + ot, mt * P : (mt + 1) * P],
                            qk_sb[po : po + HD, ot, nt * 512 : (nt + 1) * 512],
                            start=True,
                            stop=True,
                        )
                    nc.scalar.activation(
                        pT[:, mt, :], ps_s[:], AF.Exp, scale=float(HD) ** -0.5
                    )
                ps_av = ps_av_pool.tile([P, N], F32, tag="av")
                for nt in range(NT):
                    for mt in range(MT):
                        nc.tensor.matmul(
                            ps_av[0 : HD + 1, nt * 512 : (nt + 1) * 512],
                            vT_sb[:, mt, h * (HD + 1) : (h + 1) * (HD + 1)],
                            pT[:, mt, nt * 512 : (nt + 1) * 512],
                            start=(mt == 0),
                            stop=(mt == MT - 1),
                        )
                nc.vector.reciprocal(zln[:], ps_av[HD : HD + 1, :])
                nc.vector.tensor_copy(zinv[:], zln[:])
                ps_zb = ps_pool.tile([P, N], F32, tag="ps")
                for nt in range(NT):
                    nc.tensor.matmul(
                        ps_zb[0:HD, nt * 512 : (nt + 1) * 512],
                        ones64[:],
                        zinv[:, nt * 512 : (nt + 1) * 512],
                        start=True,
                        stop=True,
                    )
                nc.vector.tensor_copy(zb[:], ps_zb[0:HD, :])
                nc.vector.tensor_mul(
                    ha_sb[po : po + HD, ot, :], ps_av[0:HD, :], zb[:]
                )

            # ---------------- proj + bias + residual ----------------
            out_v = out_d.ap().rearrange("(ot p) n -> p ot n", p=P)
            for ot in range(CCH):
                ps_p = ps_pool.tile([P, N], F32, tag="ps")
                for nt in range(NT):
                    for cc in range(CCH):
                        nc.tensor.matmul(
                            ps_p[:, nt * 512 : (nt + 1) * 512],
                            wp_sb[:, cc, ot * P : (ot + 1) * P],
                            ha_sb[:, cc, nt * 512 : (nt + 1) * 512],
                            start=(cc == 0),
                            stop=(cc == CCH - 1),
                        )
                nc.vector.scalar_tensor_tensor(
                    out=out_sb[:, ot, :],
                    in0=ps_p[:],
                    scalar=bp_sb[:, ot : ot + 1],
                    in1=x_sb[:, ot, :],
                    op0=ALU.add,
                    op1=ALU.add,
                )
                nc.sync.dma_start(out_v[:, ot, :], out_sb[:, ot, :])

    nc.compile()
    return nc


def make_in_maps(x, gn_gamma, gn_beta, w_qkv, b_qkv, w_proj, b_proj):
    f32 = np.float32
    w_qkv = np.asarray(w_qkv, dtype=f32)
    b_qkv = np.asarray(b_qkv, dtype=f32)
    shared = {
        "w_qkT": np.ascontiguousarray(w_qkv[: 2 * C].T),
        "b_qk": np.ascontiguousarray(b_qkv[: 2 * C].reshape(8, P).T),
        "w_vT": np.ascontiguousarray(
            np.concatenate([w_qkv[2 * C :].T, b_qkv[2 * C :][None, :]], axis=0)
        ),
        "w_pT": np.ascontiguousarray(np.asarray(w_proj, dtype=f32).T),
        "b_p": np.ascontiguousarray(
            np.asarray(b_proj, dtype=f32).reshape(CCH, P).T
        ),
        "gamma": np.ascontiguousarray(
            np.asarray(gn_gamma, dtype=f32).reshape(CCH, P).T
        ),
        "beta": np.ascontiguousarray(
            np.asarray(gn_beta, dtype=f32).reshape(CCH, P).T
        ),
    }
    gsel = np.zeros((P, 8), f32)
    for p in range(P):
        gsel[p, p // GS] = 1.0
    shared["gsel"] = gsel
    shared["gselT"] = np.ascontiguousarray(gsel.T)
    in_maps = []
    for b in range(B):
        m = dict(shared)
        m["x"] = np.ascontiguousarray(np.asarray(x[b], dtype=f32).reshape(C, N))
        in_maps.append(m)
    return in_maps


def kernel(x, gn_gamma, gn_beta, w_qkv, b_qkv, w_proj, b_proj):
    if "nc" not in _CACHE:
        _CACHE["nc"] = build_nc()
    nc = _CACHE["nc"]
    in_maps = make_in_maps(x, gn_gamma, gn_beta, w_qkv, b_qkv, w_proj, b_proj)
    trace = bool(os.environ.get("KERNEL_TRACE"))
    res = run_bass_kernel_spmd(
        nc, in_maps, core_ids=list(range(NCORES)), trace=trace
    )
    _CACHE["last_result"] = res
    out = np.stack([np.asarray(res.results[i]["out"]) for i in range(NCORES)])
    return out.reshape(B, C, 32, 32).astype(np.float32)


# revision 13
# speedup vs baseline: 1.1869x; 1.1869x over previous
"""Trainium2 Bass kernel for nn_AttentionBlock (GroupNorm + attention block),
data-parallel over batch across 8 NeuronCores.

Reference computation (per batch element b, C=512, N=H*W=1024, heads=8, hd=64):
  xn   = GroupNorm32(x) * gamma + beta
  qkv  = w_qkv @ xn + b_qkv        (1x1 conv == matmul over channels)
  attn = softmax(q^T k / sqrt(hd)) ; ha = attn @ v ; out = x + w_proj @ ha + b_proj

Sharding: batch B=8 -> one batch element per core. No collectives.

Per-core dataflow (matmuls in bf16 with f32 PSUM accumulation; weights are
passed as f32 and converted to bf16 on chip):
  - GroupNorm stats in f32: per-channel sum / sum-of-squares on DVE, group
    reduction + per-channel re-broadcast via tiny TensorE matmuls,
    rsqrt(var+eps) computed as exp(-0.5*ln(var+eps)) to stay in a single
    ScalarE table set (with the softmax Exp).
  - q,k produced in [channel, n] layout with host-pretransposed w_qkT.
  - v produced directly TRANSPOSED, v_T[n, c_v] = xn^T @ w_v^T (host
    pretransposed rhs), so attention needs no on-chip transposes. The v bias
    is folded in via a ones-row matmul accumulation. v_T is stored with
    head stride 65: 64 v columns + a ones column per head.
  - Scores computed transposed: S_T[m, n] = k_h^T q_h (K = hd = 64).
    Softmax without max subtraction (|scores*0.125| <~ 8, exp is safe in f32):
    P_T = exp(S_T * 0.125) on ScalarE directly out of PSUM (doubles as the
    PSUM eviction), written as bf16.
  - AV: ha_u[65, n] = [v_T | ones]^T @ P_T accumulated over the 8 m-chunks;
    row 64 is the softmax denominator Z. Normalization by 1/Z is applied at
    eviction: reciprocal on DVE, broadcast over partitions via a K=1 ones
    matmul on TensorE, multiply on DVE (writing bf16 for the proj matmul).
  - proj: out = (psum + b_proj) + x fused in one DVE scalar_tensor_tensor.
"""

import os

import numpy as np

import concourse.bass as bass
import concourse.bacc as bacc
import concourse.mybir as mybir
import concourse.tile as tile
from concourse.bass_utils import run_bass_kernel_spmd

F32 = mybir.dt.float32
BF16 = mybir.dt.bfloat16
AF = mybir.ActivationFunctionType
ALU = mybir.AluOpType

B = 8
C = 512
N = 1024          # H*W = 32*32
H = 8             # num heads
HD = 64           # head dim
G = 32            # groups
GS = C // G       # channels per group = 16
CCH = 4           # channel chunks of 128
NT = 2            # n tiles of 512
MT = 8            # m tiles of 128
EPS = 1e-5
P = 128
NCORES = 8

_CACHE = {}


def build_nc():
    nc = bacc.Bacc(
        "TRN2", target_bir_lowering=False, debug=False, num_devices=NCORES
    )

    # All parameters are 2-D float32, pre-arranged on the host so every DMA
    # below is a plain contiguous copy.
    x_d = nc.declare_dram_parameter("x", [C, N], F32, isOutput=False)
    wqk_d = nc.declare_dram_parameter("w_qkT", [C, 2 * C], F32, isOutput=False)
    bqk_d = nc.declare_dram_parameter("b_qk", [P, 8], F32, isOutput=False)
    wv_d = nc.declare_dram_parameter("w_vT", [C + 1, C], F32, isOutput=False)
    wp_d = nc.declare_dram_parameter("w_pT", [C, C], F32, isOutput=False)
    bp_d = nc.declare_dram_parameter("b_p", [P, CCH], F32, isOutput=False)
    gm_d = nc.declare_dram_parameter("gamma", [P, CCH], F32, isOutput=False)
    bt_d = nc.declare_dram_parameter("beta", [P, CCH], F32, isOutput=False)
    gsel_d = nc.declare_dram_parameter("gsel", [P, 8], F32, isOutput=False)
    gselT_d = nc.declare_dram_parameter("gselT", [8, P], F32, isOutput=False)
    out_d = nc.declare_dram_parameter("out", [C, N], F32, isOutput=True)

    with tile.TileContext(nc) as tc:
        with (
            tc.tile_pool(name="singles", bufs=1) as singles,
            tc.tile_pool(name="wstage", bufs=2) as wstage,
            tc.tile_pool(name="ps", bufs=2, space="PSUM") as ps_pool,
            tc.tile_pool(name="ps_av", bufs=2, space="PSUM") as ps_av_pool,
        ):
            # ---------------- static tiles ----------------
            x_sb = singles.tile([P, CCH, N], F32)
            wqk_sb = singles.tile([P, CCH, 2 * C], BF16)
            bqk_sb = singles.tile([P, 8], F32)
            wv_sb = singles.tile([P, CCH, C], BF16)
            wvb_sb = singles.tile([1, C], BF16)
            wp_sb = singles.tile([P, CCH, C], BF16)
            bp_sb = singles.tile([P, CCH], F32)
            gm_sb = singles.tile([P, CCH], F32)
            bt_sb = singles.tile([P, CCH], F32)
            gsel_sb = singles.tile([P, 8], BF16)
            gselT_sb = singles.tile([8, P], BF16)
            gsel_st = singles.tile([P, 8], F32)
            gselT_st = singles.tile([8, P], F32)
            s12_bf = singles.tile([P, 8], BF16)
            mu_rs_bf = singles.tile([8, 8], BF16)
            ones_row = singles.tile([1, P], BF16)
            ones64 = singles.tile([1, HD], BF16)

            xn_sb = singles.tile([P, CCH, N], BF16)
            qk_sb = singles.tile([P, 8, N], BF16)       # ot 0-3: q, 4-7: k
            vT_sb = singles.tile([P, MT, H * (HD + 1)], BF16)
            pT_a = singles.tile([P, MT, N], BF16)
            pT_b = singles.tile([P, MT, N], BF16)
            ha_sb = singles.tile([P, CCH, N], BF16)
            out_sb = singles.tile([P, CCH, N], F32)

            s12_sb = singles.tile([P, 8], F32)          # cols 0-3 sum, 4-7 sumsq
            sq_scr = singles.tile([P, N], F32)
            mu_rs = singles.tile([8, 8], F32)           # cols 0-3 mu, 4-7 rs
            ex2_sb = singles.tile([8, CCH], F32)
            tmp8 = singles.tile([8, CCH], F32)
            var_sb = singles.tile([8, CCH], F32)
            lnv_sb = singles.tile([8, CCH], F32)
            s0_sb = singles.tile([P, CCH], F32)
            sbias_sb = singles.tile([P, CCH], F32)
            tmp128 = singles.tile([P, CCH], F32)
            eps_sb = singles.tile([8, 1], F32)
            zinv_t = [
                singles.tile([1, N], BF16, name=f"zinv{i}") for i in range(2)
            ]
            zln = singles.tile([1, N], F32)
            zeps_sb = singles.tile([1, 1], F32)
            zb_t = [singles.tile([HD, N], F32, name=f"zb{i}") for i in range(2)]

            # selector matrices DMA'd from host, converted to bf16 on chip
            nc.sync.dma_start(gsel_st[:], gsel_d.ap())
            nc.sync.dma_start(gselT_st[:], gselT_d.ap())
            nc.vector.tensor_copy(gsel_sb[:], gsel_st[:])
            nc.vector.tensor_copy(gselT_sb[:], gselT_st[:])
            nc.vector.memset(ones_row[:], 1.0)
            nc.vector.memset(ones64[:], 1.0)
            nc.vector.memset(eps_sb[:], EPS)
            nc.vector.memset(zeps_sb[:], 0.0)

            # ---------------- input DMAs + weight bf16 conversion ----------
            x_v = x_d.ap().rearrange("(cc p) n -> p cc n", p=P)
            for cc in range(CCH):
                nc.sync.dma_start(x_sb[:, cc, :], x_v[:, cc, :])

            wqk_st = wstage.tile([P, CCH, 2 * C], F32, tag="wst")
            nc.sync.dma_start(
                wqk_st[:], wqk_d.ap().rearrange("(cc p) o -> p cc o", p=P)
            )
            nc.scalar.copy(wqk_sb[:], wqk_st[:])

            wv_st = wstage.tile([P, CCH, C], F32, tag="wst")
            nc.sync.dma_start(
                wv_st[:], wv_d.ap()[0:C, :].rearrange("(cc p) v -> p cc v", p=P)
            )
            nc.scalar.copy(wv_sb[:], wv_st[:])
            wvb_st = wstage.tile([1, C], F32, tag="wvbst")
            nc.sync.dma_start(wvb_st[:], wv_d.ap()[C : C + 1, :])
            nc.scalar.copy(wvb_sb[:], wvb_st[:])

            wp_st = wstage.tile([P, CCH, C], F32, tag="wst")
            nc.sync.dma_start(
                wp_st[:], wp_d.ap().rearrange("(cc p) o -> p cc o", p=P)
            )
            nc.scalar.copy(wp_sb[:], wp_st[:])

            nc.sync.dma_start(bqk_sb[:], bqk_d.ap())
            nc.sync.dma_start(bp_sb[:], bp_d.ap())
            nc.sync.dma_start(gm_sb[:], gm_d.ap())
            nc.sync.dma_start(bt_sb[:], bt_d.ap())

            # ---------------- GroupNorm stats ----------------
            for cc in range(CCH):
                nc.vector.reduce_sum(
                    s12_sb[:, cc : cc + 1], x_sb[:, cc, :], axis=mybir.AxisListType.X
                )
                nc.scalar.activation(
                    sq_scr[:], x_sb[:, cc, :], AF.Square,
                    accum_out=s12_sb[:, 4 + cc : 5 + cc],
                )
            # group reduce: [8 groups-in-chunk, 8 (s1 x cc, s2 x cc)]
            nc.vector.tensor_copy(s12_bf[:], s12_sb[:])
            ps_st = ps_pool.tile([P, N], F32, tag="ps")
            nc.tensor.matmul(
                ps_st[0:8, 0:8], gsel_sb[:], s12_bf[:], start=True, stop=True
            )
            inv_cnt = 1.0 / (GS * N)
            nc.vector.tensor_scalar_mul(mu_rs[:, 0:4], ps_st[0:8, 0:4], inv_cnt)
            nc.vector.tensor_scalar_mul(ex2_sb[:], ps_st[0:8, 4:8], inv_cnt)
            nc.vector.tensor_mul(tmp8[:], mu_rs[:, 0:4], mu_rs[:, 0:4])
            nc.vector.tensor_sub(var_sb[:], ex2_sb[:], tmp8[:])
            # rsqrt(var+eps) = exp(-0.5 * ln(var+eps)); keeps one ACT table set
            nc.scalar.activation(lnv_sb[:], var_sb[:], AF.Ln, bias=eps_sb[:])
            nc.scalar.activation(mu_rs[:, 4:8], lnv_sb[:], AF.Exp, scale=-0.5)
            # broadcast group stats back to channels
            nc.vector.tensor_copy(mu_rs_bf[:], mu_rs[:])
            ps_bc = ps_pool.tile([P, N], F32, tag="ps")
            nc.tensor.matmul(
                ps_bc[0:P, 0:8], gselT_sb[:], mu_rs_bf[:], start=True, stop=True
            )
            nc.vector.tensor_mul(s0_sb[:], ps_bc[0:P, 4:8], gm_sb[:])
            nc.vector.tensor_mul(tmp128[:], ps_bc[0:P, 0:4], s0_sb[:])
            nc.vector.tensor_sub(sbias_sb[:], bt_sb[:], tmp128[:])
            # xn = x * s0 + sbias  (bf16)
            for cc in range(CCH):
                nc.vector.tensor_scalar(
                    out=xn_sb[:, cc, :],
                    in0=x_sb[:, cc, :],
                    scalar1=s0_sb[:, cc : cc + 1],
                    scalar2=sbias_sb[:, cc : cc + 1],
                    op0=ALU.mult,
                    op1=ALU.add,
                )

            # ---------------- q, k ----------------
            for ot in range(8):
                ps_qk = ps_pool.tile([P, N], F32, tag="ps")
                for nt in range(NT):
                    for cc in range(CCH):
                        nc.tensor.matmul(
                            ps_qk[:, nt * 512 : (nt + 1) * 512],
                            wqk_sb[:, cc, ot * P : (ot + 1) * P],
                            xn_sb[:, cc, nt * 512 : (nt + 1) * 512],
                            start=(cc == 0),
                            stop=(cc == CCH - 1),
                        )
                nc.scalar.activation(
                    qk_sb[:, ot, :], ps_qk[:], AF.Identity,
                    bias=bqk_sb[:, ot : ot + 1],
                )

            # ---------------- v_T (+ bias via ones row) ----------------
            for mt in range(MT):
                ps_v = ps_pool.tile([P, N], F32, tag="ps")
                for cc in range(CCH):
                    nc.tensor.matmul(
                        ps_v[:, 0:C],
                        xn_sb[:, cc, mt * P : (mt + 1) * P],
                        wv_sb[:, cc, :],
                        start=(cc == 0),
                        stop=False,
                    )
                nc.tensor.matmul(
                    ps_v[:, 0:C], ones_row[:], wvb_sb[:], start=False, stop=True
                )
                nc.scalar.copy(
                    vT_sb[:, mt, :]
                    .rearrange("p (h d) -> p h d", h=H)[:, :, 0:HD],
                    ps_v[:, 0:C].rearrange("p (h d) -> p h d", h=H),
                )
            nc.vector.memset(
                vT_sb[:].rearrange("p mt (h d) -> p mt h d", h=H)[:, :, :, HD : HD + 1],
                1.0,
            )

            # ---------------- attention, software-pipelined over heads ----
            # PE program order: S_T(h) ... AV(h-1) ... so AV's dependency
            # (exp of head h-1 on ScalarE) hides behind S_T(h)'s matmuls.
            def st_exp(h):
                pT = pT_a if h % 2 == 0 else pT_b
                po = (h % 2) * HD
                ot = h // 2
                for mt in range(MT):
                    ps_s = ps_pool.tile([P, N], F32, tag="ps", name=f"ps_s{h}_{mt}")
                    for nt in range(NT):
                        nc.tensor.matmul(
                            ps_s[:, nt * 512 : (nt + 1) * 512],
                            qk_sb[po : po + HD, 4 + ot, mt * P : (mt + 1) * P],
                            qk_sb[po : po + HD, ot, nt * 512 : (nt + 1) * 512],
                            start=True,
                            stop=True,
                        )
                    nc.scalar.activation(
                        pT[:, mt, :], ps_s[:], AF.Exp, scale=float(HD) ** -0.5
                    )

            def av_evict(h):
                pT = pT_a if h % 2 == 0 else pT_b
                po = (h % 2) * HD
                ot = h // 2
                zinv = zinv_t[h % 2]
                zb = zb_t[h % 2]
                ps_av = ps_av_pool.tile([P, N], F32, tag="av", name=f"ps_av{h}")
                for nt in range(NT):
                    for mt in range(MT):
                        nc.tensor.matmul(
                            ps_av[0 : HD + 1, nt * 512 : (nt + 1) * 512],
                            vT_sb[:, mt, h * (HD + 1) : (h + 1) * (HD + 1)],
                            pT[:, mt, nt * 512 : (nt + 1) * 512],
                            start=(mt == 0),
                            stop=(mt == MT - 1),
                        )
                nc.vector.reciprocal(zln[:], ps_av[HD : HD + 1, :])
                nc.vector.tensor_copy(zinv[:], zln[:])
                ps_zb = ps_pool.tile([P, N], F32, tag="ps", name=f"ps_zb{h}")
                for nt in range(NT):
                    nc.tensor.matmul(
                        ps_zb[0:HD, nt * 512 : (nt + 1) * 512],
                        ones64[:],
                        zinv[:, nt * 512 : (nt + 1) * 512],
                        start=True,
                        stop=True,
                    )
                nc.vector.tensor_copy(zb[:], ps_zb[0:HD, :])
                nc.vector.tensor_mul(
                    ha_sb[po : po + HD, ot, :], ps_av[0:HD, :], zb[:]
                )

            st_exp(0)
            for h in range(1, H):
                st_exp(h)
                av_evict(h - 1)
            av_evict(H - 1)

            # ---------------- proj + bias + residual ----------------
            out_v = out_d.ap().rearrange("(ot p) n -> p ot n", p=P)
            for ot in range(CCH):
                ps_p = ps_pool.tile([P, N], F32, tag="ps")
                for nt in range(NT):
                    for cc in range(CCH):
                        nc.tensor.matmul(
                            ps_p[:, nt * 512 : (nt + 1) * 512],
                            wp_sb[:, cc, ot * P : (ot + 1) * P],
                            ha_sb[:, cc, nt * 512 : (nt + 1) * 512],
                            start=(cc == 0),
                            stop=(cc == CCH - 1),
                        )
                nc.vector.scalar_tensor_tensor(
                    out=out_sb[:, ot, :],
                    in0=ps_p[:],
                    scalar=bp_sb[:, ot : ot + 1],
                    in1=x_sb[:, ot, :],
                    op0=ALU.add,
                    op1=ALU.add,
                )
                nc.sync.dma_start(out_v[:, ot, :], out_sb[:, ot, :])

    nc.compile()
    return nc


def make_in_maps(x, gn_gamma, gn_beta, w_qkv, b_qkv, w_proj, b_proj):
    f32 = np.float32
    w_qkv = np.asarray(w_qkv, dtype=f32)
    b_qkv = np.asarray(b_qkv, dtype=f32)
    shared = {
        "w_qkT": np.ascontiguousarray(w_qkv[: 2 * C].T),
        "b_qk": np.ascontiguousarray(b_qkv[: 2 * C].reshape(8, P).T),
        "w_vT": np.ascontiguousarray(
            np.concatenate([w_qkv[2 * C :].T, b_qkv[2 * C :][None, :]], axis=0)
        ),
        "w_pT": np.ascontiguousarray(np.asarray(w_proj, dtype=f32).T),
        "b_p": np.ascontiguousarray(
            np.asarray(b_proj, dtype=f32).reshape(CCH, P).T
        ),
        "gamma": np.ascontiguousarray(
            np.asarray(gn_gamma, dtype=f32).reshape(CCH, P).T
        ),
        "beta": np.ascontiguousarray(
            np.asarray(gn_beta, dtype=f32).reshape(CCH, P).T
        ),
    }
    gsel = np.zeros((P, 8), f32)
    for p in range(P):
        gsel[p, p // GS] = 1.0
    shared["gsel"] = gsel
    shared["gselT"] = np.ascontiguousarray(gsel.T)
    in_maps = []
    for b in range(B):
        m = dict(shared)
        m["x"] = np.ascontiguousarray(np.asarray(x[b], dtype=f32).reshape(C, N))
        in_maps.append(m)
    return in_maps


def kernel(x, gn_gamma, gn_beta, w_qkv, b_qkv, w_proj, b_proj):
    if "nc" not in _CACHE:
        _CACHE["nc"] = build_nc()
    nc = _CACHE["nc"]
    in_maps = make_in_maps(x, gn_gamma, gn_beta, w_qkv, b_qkv, w_proj, b_proj)
    trace = bool(os.environ.get("KERNEL_TRACE"))
    res = run_bass_kernel_spmd(
        nc, in_maps, core_ids=list(range(NCORES)), trace=trace
    )
    _CACHE["last_result"] = res
    out = np.stack([np.asarray(res.results[i]["out"]) for i in range(NCORES)])
    return out.reshape(B, C, 32, 32).astype(np.float32)
